# revision 24
# baseline (speedup 1.0000x reference)
"""DDiT block (adaLN attention + MLP) on 8 Trainium2 NeuronCores.

Sharding: cores 0-3 -> batch 0, cores 4-7 -> batch 1. Within a 4-core
batch group: attention is sharded by heads (4 heads/core, full sequence);
after the attention out-projection a grouped ReduceScatter sums the
per-head partial outputs and hands each core a 512-token slice, on which
it runs the (token-sharded) MLP.

v2 structure (vs v1 baseline):
  - adaLN LN scales A1/A2 folded into wqkv / mlp_w1 columns on host; the
    device only computes (x - mu) * rstd.
  - rstd = Exp(-0.5 * Ln(var+eps)) so the whole kernel stays in ACT's
    natural_log_exp table set (no table swaps against attention exp).
  - LN1 + transpose + QK/V projection software-pipelined per 512-token
    block.
  - scores matmuls write bf16 PSUM packed 2 key-blocks per bank; exp is
    split DVE (Schraudolph, 2x mode) / ACT (table exp).
  - softmax denominator: ones-column in V; reciprocal on DVE, broadcast
    via gpsimd partition_broadcast (no PE/psum involvement).
  - MLP1 for token chunks {0,1} interleaved into attention; MLP2 pass A
    (chunks 0,1) overlaps the last ReduceScatter; only chunk {2,3} MLP
    remains in the tail. mlp_w2 is streamed (never resident).
"""

import numpy as np

import concourse.bass as bass
import concourse.mybir as mybir
import concourse.tile as tile
from concourse import bacc
from concourse.bass_utils import run_bass_kernel_spmd
from concourse.masks import make_identity

B, S, D, H, HD = 2, 2048, 1024, 16, 64
DFF = 4 * D
TOK = S // 4          # tokens per core for the MLP phase
EPS = 1e-5
GROUPS = [[0, 1, 2, 3], [4, 5, 6, 7]]
F32 = mybir.dt.float32
BF16 = mybir.dt.bfloat16
I16 = mybir.dt.int16
AF = mybir.ActivationFunctionType
ALU = mybir.AluOpType

_CACHE = {}


# ---------------------------------------------------------------- host prep

def _f(v):
    return np.ascontiguousarray(np.asarray(v, dtype=np.float32))


def _bf(a):
    import ml_dtypes
    return np.ascontiguousarray(a.astype(ml_dtypes.bfloat16))


def host_prep(inp):
    x, c = _f(inp["x"]), _f(inp["c"])
    norm1_w, norm2_w = _f(inp["norm1_w"]), _f(inp["norm2_w"])
    w_qkv, w_out = _f(inp["w_qkv"]), _f(inp["w_out"])
    mlp_w1, mlp_b1 = _f(inp["mlp_w1"]), _f(inp["mlp_b1"])
    mlp_w2, mlp_b2 = _f(inp["mlp_w2"]), _f(inp["mlp_b2"])
    ada_w, ada_b = _f(inp["ada_w"]), _f(inp["ada_b"])

    ada = c @ ada_w.T + ada_b                      # [B, 6D]
    sh_msa, sc_msa, g_msa, sh_mlp, sc_mlp, g_mlp = np.split(ada, 6, axis=1)
    A1 = norm1_w[None] * (1.0 + sc_msa)            # [B, D]
    A2 = norm2_w[None] * (1.0 + sc_mlp)
    bias_qkv = sh_msa @ w_qkv.T                    # [B, 3D]
    bias1 = mlp_b1[None] + sh_mlp @ mlp_w1.T       # [B, DFF]

    wq, wk, wv = w_qkv[0:D], w_qkv[D:2 * D], w_qkv[2 * D:3 * D]

    # per-batch A2-folded w1 in [rt, p, kc*128+r] layout (contiguous DMA)
    w1blk_b, w2T_b, xres_b = [], [], []
    for b in range(B):
        w1A = mlp_w1 * A2[b][None, :]              # [DFF, D]
        # w1blk[rt, p, kc*128 + r] = w1A[rt*128+r, kc*128+p]
        blk = w1A.reshape(32, 128, 8, 128)         # [rt, r, kc, p]
        blk = np.ascontiguousarray(blk.transpose(0, 3, 2, 1))  # [rt, p, kc, r]
        w1blk_b.append(_bf(blk.reshape(32, 128, 8 * 128)))
        w2g = g_mlp[b][:, None] * mlp_w2           # [D, DFF]
        w2T_b.append(_bf(w2g.T.copy()))            # [DFF, D]
        xres_b.append(_bf(x[b] + (g_mlp[b] * mlp_b2)[None, :]))

    in_maps = []
    for cid in range(8):
        b, r = cid // 4, cid % 4
        hsl = slice(256 * r, 256 * r + 256)
        woutg = g_msa[b][:, None] * w_out          # [D, D]
        wqkT = np.vstack([wq[hsl], wk[hsl]]).T * A1[b][:, None]   # [D, 512]
        wvT = wv[hsl].T * A1[b][:, None]                          # [D, 256]
        in_maps.append({
            "x_b": _bf(x[b]),
            "x_res": np.ascontiguousarray(np.concatenate(
                [xres_b[b][512 * t2 + 128 * r:512 * t2 + 128 * r + 128]
                 for t2 in range(4)])),
            "wqkT": _bf(wqkT),
            "bias_qk": np.ascontiguousarray(np.concatenate(
                [bias_qkv[b, hsl],
                 bias_qkv[b, D + 256 * r:D + 256 * r + 256]])),    # [512]
            "wvT": _bf(wvT),
            "bias_v": np.ascontiguousarray(
                bias_qkv[b, 2 * D + 256 * r:2 * D + 256 * r + 256]),
            "woutT": _bf(woutg[:, hsl].T.copy()),                  # [256, D]
            "w1blk": w1blk_b[b],                                   # [32,128,1024]
            "bias1": np.ascontiguousarray(bias1[b]),
            "w2gT": w2T_b[b],                                      # [DFF, D]
        })
    return in_maps


# ------------------------------------------------------------- device build

def _bc(ap, p=128):
    """Broadcast a DRAM row AP across p partitions (stride-0 partition dim)."""
    return bass.AP(tensor=ap.tensor, offset=ap.offset,
                   ap=[[0, p]] + [list(d) for d in ap.ap])


def build_program(reps=1):
    nc = bacc.Bacc("TRN2", target_bir_lowering=False, debug=False, num_devices=8)

    x_d = nc.dram_tensor("x_b", [S, D], BF16, kind="ExternalInput")
    xr_d = nc.dram_tensor("x_res", [TOK, D], BF16, kind="ExternalInput")
    wqk_d = nc.dram_tensor("wqkT", [D, 512], BF16, kind="ExternalInput")
    bqk_d = nc.dram_tensor("bias_qk", [512], F32, kind="ExternalInput")
    wv_d = nc.dram_tensor("wvT", [D, 256], BF16, kind="ExternalInput")
    bv_d = nc.dram_tensor("bias_v", [256], F32, kind="ExternalInput")
    wo_d = nc.dram_tensor("woutT", [256, D], BF16, kind="ExternalInput")
    w1_d = nc.dram_tensor("w1blk", [32, 128, 8 * 128], BF16, kind="ExternalInput")
    b1_d = nc.dram_tensor("bias1", [DFF], F32, kind="ExternalInput")
    w2_d = nc.dram_tensor("w2gT", [DFF, D], BF16, kind="ExternalInput")
    out_d = nc.dram_tensor("out", [TOK, D], F32, kind="ExternalOutput")

    with tile.TileContext(nc, num_cores=8) as tc:
        for _ in range(reps):
            _body(nc, tc, x_d, xr_d, wqk_d, bqk_d, wv_d, bv_d,
                  wo_d, w1_d, b1_d, w2_d, out_d)
    nc.compile()
    return nc


def _ln_rstd_act(nc, pool, mv, eps_t, tag):
    """rstd = 1/sqrt(var + eps) via ACT Sqrt + DVE reciprocal (sqrt set)."""
    rstd = pool.tile([128, 1], F32, tag=f"rstd{tag}", name=f"rstd{tag}")
    nc.scalar.activation(out=rstd, in_=mv[:, 1:2], func=AF.Sqrt, bias=eps_t,
                         scale=1.0)
    nc.vector.reciprocal(out=rstd, in_=rstd)
    return rstd


def _ln_rstd_dve(nc, pool, mv, tag):
    """DVE-only fast rsqrt (magic constant + 1 Newton step); no ACT table.
    Used for the LN2 chunks that land mid-attention, where an ACT Sqrt
    would force activation-table swaps against the softmax Exp."""
    I32 = mybir.dt.int32
    t = pool.tile([128, 1], F32, tag=f"t{tag}", name=f"t{tag}")
    nc.vector.tensor_scalar(out=t, in0=mv[:, 1:2], scalar1=EPS, scalar2=1.0,
                            op0=ALU.add, op1=ALU.mult)
    si = pool.tile([128, 1], I32, tag=f"si{tag}", name=f"si{tag}")
    nc.vector.tensor_scalar(out=si, in0=t[:].bitcast(I32), scalar1=1,
                            scalar2=0, op0=ALU.logical_shift_right,
                            op1=ALU.logical_shift_right)
    yi = pool.tile([128, 1], I32, tag=f"yi{tag}", name=f"yi{tag}")
    nc.vector.tensor_scalar(out=yi, in0=si, scalar1=-1,
                            scalar2=0x5F3759DF, op0=ALU.mult, op1=ALU.add)
    y0 = yi[:].bitcast(F32)
    a = pool.tile([128, 1], F32, tag=f"a{tag}", name=f"a{tag}")
    nc.vector.tensor_tensor(out=a, in0=y0, in1=y0, op=ALU.mult)
    b = pool.tile([128, 1], F32, tag=f"b{tag}", name=f"b{tag}")
    nc.vector.tensor_tensor(out=b, in0=a, in1=t, op=ALU.mult)
    c = pool.tile([128, 1], F32, tag=f"c{tag}", name=f"c{tag}")
    nc.vector.tensor_scalar(out=c, in0=b, scalar1=-0.5, scalar2=1.5,
                            op0=ALU.mult, op1=ALU.add)
    rstd = pool.tile([128, 1], F32, tag=f"rstd{tag}", name=f"rstd{tag}")
    nc.vector.tensor_tensor(out=rstd, in0=y0, in1=c, op=ALU.mult)
    return rstd


def _body(nc, tc, x_d, xr_d, wqk_d, bqk_d, wv_d, bv_d,
          wo_d, w1_d, b1_d, w2_d, out_d):
    mm = nc.tensor.matmul

    from contextlib import ExitStack
    with ExitStack() as outer:
        consts = outer.enter_context(tc.tile_pool(name="consts", bufs=1))
        pers = outer.enter_context(tc.tile_pool(name="pers", bufs=1))
        dram = outer.enter_context(tc.tile_pool(name="dram", bufs=1, space="DRAM"))

        # ---- constants
        ident = consts.tile([128, 128], BF16, tag="ident", name="ident")
        make_identity(nc, ident)
        eps_t = consts.tile([128, 1], F32, tag="eps", name="eps")
        nc.vector.memset(eps_t, EPS)
        bqk_t = consts.tile([128, 4], F32, tag="bqk", name="bqk")
        nc.sync.dma_start(out=bqk_t, in_=bass.AP(
            tensor=bqk_d[:].tensor, offset=0, ap=[[1, 128], [128, 4]]))
        b1_t = consts.tile([128, 32], F32, tag="b1t", name="b1t")
        nc.sync.dma_start(out=b1_t, in_=bass.AP(
            tensor=b1_d[:].tensor, offset=0, ap=[[1, 128], [128, 32]]))
        bvbc = consts.tile([128, 256], F32, tag="bvbc", name="bvbc")
        nc.sync.dma_start(out=bvbc, in_=_bc(bv_d[:]))

        # ---- persistent activations
        qkT = [pers.tile([128, S], BF16, tag=f"qkT{rt}", name=f"qkT{rt}")
               for rt in range(4)]
        v_aug = [pers.tile([128, 4, 65], BF16, tag=f"vaug{t}", name=f"vaug{t}")
                 for t in range(16)]
        h2T = pers.tile([128, 8, TOK], BF16, tag="h2T", name="h2T")
        x2 = [pers.tile([128, D], BF16, tag=f"x2_{t}", name=f"x2_{t}")
              for t in range(4)]
        g1T = [pers.tile([128, TOK], BF16, tag=f"g1T{rt}", name=f"g1T{rt}")
               for rt in range(32)]
        wo_sb = [pers.tile([128, D], BF16, tag=f"wo{k}", name=f"wo{k}")
                 for k in range(2)]
        for kc in range(2):
            nc.sync.dma_start(out=wo_sb[kc], in_=wo_d[kc * 128:(kc + 1) * 128, :])

        # ---- DRAM scratch for the chunked collective
        y_part = [dram.tile([512, D], BF16, tag=f"y_part{i}", name=f"y_part{i}")
                  for i in range(4)]
        y_sum = [dram.tile([128, D], BF16, tag=f"y_sum{i}", name=f"y_sum{i}")
                 for i in range(4)]

        # =================== P1-P3: LN1 + transpose + QKV =====================
        with ExitStack() as early:
            wep = early.enter_context(tc.tile_pool(name="wep", bufs=1))
            hTp = early.enter_context(tc.tile_pool(name="hTp", bufs=1))
            lnp = early.enter_context(tc.tile_pool(name="lnp", bufs=2))
            psT = early.enter_context(
                tc.tile_pool(name="psT", bufs=2, space="PSUM"))
            psQK = early.enter_context(
                tc.tile_pool(name="psQK", bufs=2, space="PSUM"))
            psV = early.enter_context(
                tc.tile_pool(name="psV", bufs=2, space="PSUM"))

            wqk_sb = [wep.tile([128, 512], BF16, tag=f"wqk{k}", name=f"wqk{k}")
                      for k in range(8)]
            wv_sb = [wep.tile([128, 256], BF16, tag=f"wv{k}", name=f"wv{k}")
                     for k in range(8)]
            for kc in range(8):
                nc.sync.dma_start(out=wqk_sb[kc],
                                  in_=wqk_d[kc * 128:(kc + 1) * 128, :])
                nc.sync.dma_start(out=wv_sb[kc],
                                  in_=wv_d[kc * 128:(kc + 1) * 128, :])

            hT = hTp.tile([128, 8, S], BF16, tag="hT", name="hT")

            xtp = early.enter_context(tc.tile_pool(name="xtp", bufs=4))
            for tb in range(4):
                for tt in range(tb * 4, tb * 4 + 4):
                    xt = xtp.tile([128, D], BF16, tag="xt", name="xt")
                    nc.sync.dma_start(out=xt, in_=x_d[tt * 128:(tt + 1) * 128, :])
                    st = lnp.tile([128, 2, 6], F32, tag="st", name="st")
                    xg = xt.rearrange("p (g d) -> p g d", g=2)
                    for g in range(2):
                        nc.vector.bn_stats(out=st[:, g, :], in_=xg[:, g, :])
                    mv = lnp.tile([128, 2], F32, tag="mv", name="mv")
                    nc.vector.bn_aggr(out=mv, in_=st)
                    rstd = _ln_rstd_act(nc, lnp, mv, eps_t, "1")
                    ht = lnp.tile([128, D], BF16, tag="ht", name="ht")
                    if tt % 2 == 0:
                        nc.vector.tensor_scalar(out=ht, in0=xt,
                                                scalar1=mv[:, 0:1],
                                                scalar2=rstd, op0=ALU.subtract,
                                                op1=ALU.mult)
                    else:
                        nmr = lnp.tile([128, 1], F32, tag="nmr", name="nmr")
                        nc.vector.tensor_scalar(out=nmr, in0=mv[:, 0:1],
                                                scalar1=rstd, scalar2=-1.0,
                                                op0=ALU.mult, op1=ALU.mult)
                        nc.scalar.activation(out=ht, in_=xt, func=AF.Identity,
                                             bias=nmr, scale=rstd)
                    for dcg in range(2):
                        pt = psT.tile([128, 4, 128], BF16, tag="pt", name="pt")
                        for j in range(4):
                            dc = 4 * dcg + j
                            nc.tensor.transpose(
                                pt[:, j, :], ht[:, dc * 128:(dc + 1) * 128],
                                ident)
                        dst = hT[:, 4 * dcg:4 * dcg + 4,
                                 tt * 128:(tt + 1) * 128]
                        if (tt + dcg) % 2 == 0:
                            nc.vector.tensor_copy(out=dst, in_=pt)
                        else:
                            nc.scalar.copy(out=dst, in_=pt)

                # QK projection for this token block (feature-major out)
                tsl = slice(tb * 512, (tb + 1) * 512)
                for rt in range(4):
                    pm = psQK.tile([128, 512], F32, tag="pm", name="pm")
                    for kc in range(8):
                        mm(pm, lhsT=wqk_sb[kc][:, rt * 128:(rt + 1) * 128],
                           rhs=hT[:, kc, tsl], start=(kc == 0), stop=(kc == 7))
                    if rt % 2 == 0:
                        nc.vector.tensor_scalar_add(
                            out=qkT[rt][:, tsl], in0=pm,
                            scalar1=bqk_t[:, rt:rt + 1])
                    else:
                        nc.scalar.activation(
                            out=qkT[rt][:, tsl], in_=pm, func=AF.Identity,
                            bias=bqk_t[:, rt:rt + 1], scale=1.0)

                # V projection (token-major out) for the 4 tiles of this block
                for tt in range(tb * 4, tb * 4 + 4):
                    pv = psV.tile([128, 256], F32, tag="pmv", name="pmv")
                    for kc in range(8):
                        mm(pv, lhsT=hT[:, kc, tt * 128:(tt + 1) * 128],
                           rhs=wv_sb[kc], start=(kc == 0), stop=(kc == 7))
                    nc.vector.memset(v_aug[tt][:, :, 64:65], 1.0)
                    nc.vector.tensor_tensor(
                        out=v_aug[tt][:, :, 0:64],
                        in0=pv.rearrange("p (h d) -> p h d", h=4),
                        in1=bvbc.rearrange("p (h d) -> p h d", h=4),
                        op=ALU.add)

        # =================== attention + interleaved MLP ======================
        with tc.tile_pool(name="w1p", bufs=1) as w1p, \
             tc.tile_pool(name="attp", bufs=2) as attp, \
             tc.tile_pool(name="mstr", bufs=2) as mstr, \
             tc.tile_pool(name="psMix", bufs=2, space="PSUM") as psMix:

            w1_sb = [w1p.tile([128, 8 * 128], BF16, tag=f"w1_{rt}",
                              name=f"w1_{rt}") for rt in range(32)]
            for rt in range(32):
                nc.sync.dma_start(out=w1_sb[rt], in_=w1_d[rt])

            def ln2_chunk(t2):
                ys = mstr.tile([128, D], BF16, tag="ys", name="ys")
                nc.sync.dma_start(out=ys, in_=y_sum[t2][:])
                xr = mstr.tile([128, D], BF16, tag="xr", name="xr")
                nc.sync.dma_start(out=xr, in_=xr_d[t2 * 128:(t2 + 1) * 128, :])
                nc.vector.tensor_tensor(out=x2[t2], in0=xr, in1=ys, op=ALU.add)
                st2 = mstr.tile([128, 2, 6], F32, tag="st2", name="st2")
                xg2 = x2[t2].rearrange("p (g d) -> p g d", g=2)
                for g in range(2):
                    nc.vector.bn_stats(out=st2[:, g, :], in_=xg2[:, g, :])
                mv2 = mstr.tile([128, 2], F32, tag="mv2", name="mv2")
                nc.vector.bn_aggr(out=mv2, in_=st2)
                rstd2 = _ln_rstd_dve(nc, mstr, mv2, "2")
                h2 = mstr.tile([128, D], BF16, tag="h2", name="h2")
                nc.vector.tensor_scalar(out=h2, in0=x2[t2], scalar1=mv2[:, 0:1],
                                        scalar2=rstd2, op0=ALU.subtract,
                                        op1=ALU.mult)
                for dcg in range(2):
                    pt2 = psMix.tile([128, 4, 128], BF16, tag="mix", name="pt2")
                    for j in range(4):
                        dc = 4 * dcg + j
                        nc.tensor.transpose(
                            pt2[:, j, :], h2[:, dc * 128:(dc + 1) * 128], ident)
                    dst = h2T[:, 4 * dcg:4 * dcg + 4, t2 * 128:(t2 + 1) * 128]
                    if dcg == 0:
                        nc.vector.tensor_copy(out=dst, in_=pt2)
                    else:
                        nc.scalar.copy(out=dst, in_=pt2)

            def mlp1_half(half, defer_gelu):
                csl = slice(half * 256, half * 256 + 256)
                for rt in range(32):
                    pm1 = psMix.tile([128, 256], F32, tag="mix", name="pm1")
                    for kc in range(8):
                        mm(pm1, lhsT=w1_sb[rt][:, kc * 128:(kc + 1) * 128],
                           rhs=h2T[:, kc, csl], start=(kc == 0), stop=(kc == 7))
                    if defer_gelu:
                        # stay in the exp table set mid-attention: evac with
                        # Identity(+bias); gelu applied in place later
                        if rt % 2 == 0:
                            nc.vector.tensor_scalar_add(
                                out=g1T[rt][:, csl], in0=pm1,
                                scalar1=b1_t[:, rt:rt + 1])
                        else:
                            nc.scalar.activation(out=g1T[rt][:, csl], in_=pm1,
                                                 func=AF.Identity,
                                                 bias=b1_t[:, rt:rt + 1],
                                                 scale=1.0)
                    else:
                        nc.scalar.activation(out=g1T[rt][:, csl], in_=pm1,
                                             func=AF.Gelu_apprx_tanh,
                                             bias=b1_t[:, rt:rt + 1], scale=1.0)

            with tc.tile_pool(name="psS", bufs=2, space="PSUM") as psS, \
                 tc.tile_pool(name="psN", bufs=2, space="PSUM") as psN, \
                 tc.tile_pool(name="expp", bufs=1) as expp, \
                 tc.tile_pool(name="divp", bufs=2) as divp, \
                 tc.tile_pool(name="ybp", bufs=2) as ybp:

                att = None
                for qb in range(4):
                    qsl = slice(qb * 512, (qb + 1) * 512)
                    att = [attp.tile([128, 512], BF16, tag=f"att{k}",
                                     name=f"att{k}") for k in range(2)]
                    for pair in range(2):
                        exps = {}
                        for kt in range(16):
                            pss = {}
                            for sub in range(2):
                                psl = slice(sub * 64, sub * 64 + 64)
                                ps = psS.tile([128, 512], F32,
                                              tag=f"sc{sub}", name=f"sc{sub}")
                                pss[sub] = ps
                                mm(ps,
                                   lhsT=qkT[2 + pair][psl,
                                                      kt * 128:(kt + 1) * 128],
                                   rhs=qkT[pair][psl, qsl],
                                   start=True, stop=True,
                                   tile_position=(sub * 64, 0))
                            for sub in range(2):
                                ps = pss[sub]
                                if kt % 2 == 0:
                                    ei = expp.tile([128, 512], I16,
                                                   tag=f"e{sub}_{kt}",
                                                   name=f"e{sub}_{kt}")
                                    nc.vector.tensor_scalar(
                                        out=ei, in0=ps, scalar1=23.083128,
                                        scalar2=16250.5, op0=ALU.mult,
                                        op1=ALU.add)
                                    exps[(sub, kt)] = ei[:].bitcast(BF16)
                                else:
                                    ex = expp.tile([128, 512], BF16,
                                                   tag=f"e{sub}_{kt}",
                                                   name=f"e{sub}_{kt}")
                                    nc.scalar.activation(out=ex, in_=ps,
                                                         func=AF.Exp,
                                                         scale=0.125)
                                    exps[(sub, kt)] = ex
                        # numerators + division for the two heads of this pair
                        for sub in range(2):
                            h = 2 * pair + sub
                            pn = psN.tile([128, 512], F32, tag="num",
                                          name="num")
                            for kc in range(16):
                                mm(pn[0:65, :], lhsT=v_aug[kc][:, h, :],
                                   rhs=exps[(sub, kc)],
                                   start=(kc == 0), stop=(kc == 15))
                            rc = divp.tile([1, 512], BF16, tag="rc", name="rc")
                            with nc.allow_low_precision(reason="bf16 denom"):
                                nc.vector.reciprocal(out=rc, in_=pn[64:65, :])
                            rcb = divp.tile([64, 512], BF16, tag="rcb",
                                            name="rcb")
                            nc.gpsimd.partition_broadcast(rcb[:], rc[:])
                            if h % 2 == 0:
                                nc.vector.tensor_tensor(
                                    out=att[h // 2][0:64, :], in0=pn[0:64, :],
                                    in1=rcb, op=ALU.mult)
                            else:
                                ad = divp.tile([64, 512], BF16, tag="adiv",
                                               name="adiv")
                                nc.vector.tensor_tensor(out=ad, in0=pn[0:64, :],
                                                        in1=rcb, op=ALU.mult)
                                nc.sync.dma_start(out=att[h // 2][64:128, :],
                                                  in_=ad)
                    # out-projection for this q-block (token-major partial y);
                    # shares the numerator pool's two PSUM banks
                    for tt in range(4):
                        yb = ybp.tile([128, D], BF16, tag="ysb", name="ysb")
                        for n in range(2):
                            po = psN.tile([128, 512], F32, tag="num", name="po")
                            for kc in range(2):
                                mm(po,
                                   lhsT=att[kc][:, tt * 128:(tt + 1) * 128],
                                   rhs=wo_sb[kc][:, n * 512:(n + 1) * 512],
                                   start=(kc == 0), stop=(kc == 1))
                            if n == 0:
                                nc.vector.tensor_copy(out=yb[:, 0:512], in_=po)
                            else:
                                nc.scalar.copy(out=yb[:, 512:1024], in_=po)
                        nc.sync.dma_start(
                            out=y_part[qb][tt * 128:(tt + 1) * 128, :], in_=yb)
                    nc.gpsimd.collective_compute(
                        "ReduceScatter", ALU.add, replica_groups=GROUPS,
                        ins=[y_part[qb].opt()], outs=[y_sum[qb].opt()])
                    if qb >= 1:
                        ln2_chunk(qb - 1)
                    if qb == 2:
                        mlp1_half(0, defer_gelu=True)

            # ======== MLP2 pass A (chunks 0,1) overlaps the last RS ==========
            with tc.tile_pool(name="psM2", bufs=1, space="PSUM") as psM2, \
                 tc.tile_pool(name="w2s", bufs=3) as w2s, \
                 tc.tile_pool(name="obp", bufs=2) as obp:

                def mlp2_pass(t2s, tag):
                    pm2 = {}
                    for t2 in t2s:
                        for n in range(2):
                            pm2[(t2, n)] = psM2.tile(
                                [128, 512], F32, tag=f"m2_{t2 % 2}_{n}",
                                name=f"m2_{t2}_{n}")
                    for kc in range(32):
                        w2t = w2s.tile([128, D], BF16, tag="w2t", name="w2t")
                        nc.sync.dma_start(
                            out=w2t, in_=w2_d[kc * 128:(kc + 1) * 128, :])
                        for t2 in t2s:
                            for n in range(2):
                                mm(pm2[(t2, n)],
                                   lhsT=g1T[kc][:, t2 * 128:(t2 + 1) * 128],
                                   rhs=w2t[:, n * 512:(n + 1) * 512],
                                   start=(kc == 0), stop=(kc == 31))
                    for t2 in t2s:
                        ob = obp.tile([128, D], F32, tag="ob", name="ob")
                        for n in range(2):
                            nsl = slice(n * 512, (n + 1) * 512)
                            nc.vector.tensor_tensor(
                                out=ob[:, nsl], in0=pm2[(t2, n)],
                                in1=x2[t2][:, nsl], op=ALU.add)
                        nc.sync.dma_start(
                            out=out_d[t2 * 128:(t2 + 1) * 128, :], in_=ob)

                # deferred gelu for chunk {0,1}, in place, batched. The zero
                # "gate" bias reads the last attention tile, so none of these
                # can be scheduled before attention's exps finish (keeps the
                # ACT table-set swap count at one).
                gate = mstr.tile([128, 1], F32, tag="gate", name="gate")
                nc.vector.tensor_scalar(out=gate, in0=att[1][:, 511:512],
                                        scalar1=0.0, scalar2=0.0, op0=ALU.mult,
                                        op1=ALU.mult)
                for rt in range(32):
                    nc.scalar.activation(out=g1T[rt][:, 0:256],
                                         in_=g1T[rt][:, 0:256],
                                         func=AF.Gelu_apprx_tanh, bias=gate,
                                         scale=1.0)
                mlp2_pass((0, 1), "a")
                ln2_chunk(3)
                mlp1_half(1, defer_gelu=False)
                mlp2_pass((2, 3), "b")


# ----------------------------------------------------------------- kernel()

def _get_nc():
    if "nc" not in _CACHE:
        _CACHE["nc"] = build_program()
    return _CACHE["nc"]


def kernel(**inputs) -> np.ndarray:
    in_maps = host_prep(inputs)
    nc = _get_nc()
    res = run_bass_kernel_spmd(nc, in_maps, list(range(8)))
    out = np.zeros((B, S, D), np.float32)
    for cid in range(8):
        b, r = cid // 4, cid % 4
        o = res.results[cid]["out"]
        for t2 in range(4):
            out[b, 512 * t2 + 128 * r:512 * t2 + 128 * r + 128] = \
                o[128 * t2:128 * t2 + 128]
    return out
